# revision 3
# baseline (speedup 1.0000x reference)
"""CLIP (InfoNCE) loss kernel for Trainium2, 8 NeuronCores.

loss = 0.5*(ce_m + ce_s) where
  ce_m = mean_i( LSE_j(l[i,:]) - l[i,i] ),  ce_s = mean_j( LSE_i(l[:,j]) - l[j,j] )
  l = logit_scale * (m @ s.T),  B=16384, D=256.

Data parallel on batch rows, 8 cores; core c owns rows [c*2048, (c+1)*2048)
of m and sees the full s.

Per core:
  - Features are quantized (UNSCALED) to fp8 e4m3 in a k-interleaved layout
    [128, 2, N]; main logits tiles use DoubleRow fp8 matmuls (K=256 fused in
    one PE pass, ~2x bf16 FLOPs).  logit_scale is applied inside the ACT
    affine (exp(scale*l - shift)), so quantization never clips.
  - mt-outer / g-inner tiling: 16 row-tiles x 8 column groups of [128, 2048].
    One ScalarE exp per group tile (PSUM f32 -> SBUF bf16) with fused
    accum_out producing the per-row partial sums (computed pre-rounding in
    f32) -- ScalarE is the bottleneck engine and runs ~94% occupied.
  - Column sums: per-group bf16 accumulators acc_g += E on DVE (idle
    otherwise); at mt=0 ACT writes acc_g directly.  After the last row-tile,
    ones-vector matmuls reduce each acc_g across partitions, batched 3 groups
    per borrowed PSUM slot on PE column strips 0/32/64 (quad 3 unusable).
  - diag l[i,i] is exact f32: row-dot of natural-layout scaled shards
    (DVE mul+reduce), spread through the main loop.
  - host merges per-core partials in float64:
      rowLSE = SHIFT + log(rowsum); colLSE = SHIFT + log(sum_c colsum_c)
      loss = mean(0.5*(rowLSE + colLSE) - diag)

SHIFT = 6*|scale|*sqrt(D) (a ~6-sigma bound on logits ~ N(0, scale^2 D)):
exp never overflows, and underflow to 0 only hits terms ~e^-80 below the
row/col max -- far below f32 relative precision.  fp8 quantization of the
inputs perturbs the loss by ~7e-4 relative (tolerance 2e-2): LSE is
max-dominated, the exact-diag term is computed in f32, and quantization
noise on 256-term dots is ~0.5 absolute on logits with sigma=16.
"""

import math
from contextlib import ExitStack

import numpy as np
import ml_dtypes

import concourse.bacc as bacc
import concourse.tile as tile
from concourse import mybir
from concourse.bass_utils import run_bass_kernel_spmd

FP8 = ml_dtypes.float8_e4m3

B = 16384
D = 256
NCORES = 8
ROWS = B // NCORES          # 2048 rows per core
P = 128
MT = ROWS // P              # 16 row-tiles
KC = D // P                 # 2 k-chunks (fused by DoubleRow)
W = 2048                    # column group width (4 psum banks f32)
GN = B // W                 # 8 column groups
SUBW = 512                  # matmul free dim (one psum bank)
NSUB = W // SUBW            # 4

f32 = mybir.dt.float32
bf16 = mybir.dt.bfloat16
fp8 = mybir.dt.float8e4

_nc_cache: dict = {}


def _build(shift: float, scale: float) -> "bacc.Bacc":
    nc = bacc.Bacc(trn_type="TRN2")

    m8_d = nc.dram_tensor("m8", [P, KC, ROWS], fp8, kind="ExternalInput")
    s8_d = nc.dram_tensor("s8", [P, KC, B], fp8, kind="ExternalInput")
    mnat_d = nc.dram_tensor("mnat", [ROWS, D], f32, kind="ExternalInput")
    snat_d = nc.dram_tensor("snat", [ROWS, D], f32, kind="ExternalInput")

    rowsum_d = nc.dram_tensor("rowsum", [P, MT], f32, kind="ExternalOutput")
    diag_d = nc.dram_tensor("diag", [P, MT], f32, kind="ExternalOutput")
    colsum_d = nc.dram_tensor("colsum", [GN, W], f32, kind="ExternalOutput")

    with ExitStack() as ctx:
        tc = ctx.enter_context(tile.TileContext(nc))
        singles = ctx.enter_context(tc.tile_pool(name="singles", bufs=1))
        epool = ctx.enter_context(tc.tile_pool(name="epool", bufs=6))
        diagpool = ctx.enter_context(tc.tile_pool(name="diagpool", bufs=4))
        mainps = ctx.enter_context(tc.tile_pool(name="mainps", bufs=2, space="PSUM"))

        # ramp: first row-tile of m8 and first chunk of s8[0] land first
        m8_sb = singles.tile([P, KC, ROWS], fp8, tag="m8")
        nc.sync.dma_start(out=m8_sb[:, :, 0:P], in_=m8_d[:, :, 0:P])
        s8_sb = [
            singles.tile([P, KC, W], fp8, name=f"s8_{g}", tag=f"s8_{g}")
            for g in range(GN)
        ]
        for q in range(4):
            nc.sync.dma_start(
                out=s8_sb[0][:, :, q * SUBW : (q + 1) * SUBW],
                in_=s8_d[:, :, q * SUBW : (q + 1) * SUBW],
            )
        nc.sync.dma_start(out=m8_sb[:, :, P:ROWS], in_=m8_d[:, :, P:ROWS])
        for g in range(1, GN):
            nc.sync.dma_start(out=s8_sb[g], in_=s8_d[:, :, g * W : (g + 1) * W])

        ones = singles.tile([P, 1], bf16, tag="ones")
        nc.vector.memset(ones, 1.0)
        negshift = singles.tile([P, 1], f32, tag="negshift")
        nc.vector.memset(negshift, -shift)

        rowsums_sb = singles.tile([P, MT * GN], f32, tag="rowsums")
        rowfinal = singles.tile([P, MT], f32, tag="rowfinal")
        colsum_sb = singles.tile([P, 3 * W], f32, tag="colsum")
        diagfinal = singles.tile([P, MT], f32, tag="diagfinal")
        accs = [
            singles.tile([P, W], bf16, name=f"acc_{g}", tag=f"acc_{g}")
            for g in range(GN)
        ]

        for mt in range(MT):
            for g in range(GN):
                ps = mainps.tile([P, W], f32, tag="ps")
                for sub in range(NSUB):
                    nc.tensor.matmul(
                        ps[:, sub * SUBW : (sub + 1) * SUBW],
                        lhsT=m8_sb[:, :, mt * P : (mt + 1) * P],
                        rhs=s8_sb[g][:, :, sub * SUBW : (sub + 1) * SUBW],
                        start=True,
                        stop=True,
                        perf_mode=mybir.MatmulPerfMode.DoubleRow,
                    )
                slot = mt * GN + g
                # ~30% of row-tiles compute their rowsum on DVE instead of
                # the ACT accumulator: balances ACT (bottleneck, -187ns per
                # offloaded instr) against DVE slack (+2.6us per reduce)
                offload = mt > 0 and (slot * 997) % 128 < 38
                if mt == 0:
                    nc.scalar.activation(
                        accs[g], ps, mybir.ActivationFunctionType.Exp,
                        bias=negshift[:, 0:1], scale=scale,
                        accum_out=rowsums_sb[:, slot : slot + 1],
                    )
                elif offload:
                    e = epool.tile([P, W], bf16)
                    nc.scalar.activation(
                        e, ps, mybir.ActivationFunctionType.Exp,
                        bias=negshift[:, 0:1], scale=scale,
                    )
                    nc.vector.reduce_sum(
                        rowsums_sb[:, slot : slot + 1], e,
                        axis=mybir.AxisListType.X,
                    )
                    nc.vector.tensor_add(accs[g], e, accs[g])
                else:
                    e = epool.tile([P, W], bf16)
                    nc.scalar.activation(
                        e, ps, mybir.ActivationFunctionType.Exp,
                        bias=negshift[:, 0:1], scale=scale,
                        accum_out=rowsums_sb[:, slot : slot + 1],
                    )
                    nc.vector.tensor_add(accs[g], e, accs[g])
                if mt == MT - 1 and g in (2, 5, 7):
                    # colsum batch: acc_g final after add(15, g); borrow a
                    # mainps slot, one PE column strip (0/32/64) per group
                    batch = {2: (0, 1, 2), 5: (3, 4, 5), 7: (6, 7)}[g]
                    b = {2: 0, 5: 1, 7: 2}[g]
                    colps = mainps.tile([P, W], f32, tag="ps", name="colps")
                    for idx, gg in enumerate(batch):
                        row = 32 * idx
                        for sub in range(NSUB):
                            nc.tensor.matmul(
                                colps[row : row + 1, sub * SUBW : (sub + 1) * SUBW],
                                lhsT=ones,
                                rhs=accs[gg][:, sub * SUBW : (sub + 1) * SUBW],
                                start=True,
                                stop=True,
                            )
                    nb = len(batch)
                    hi = 32 * (nb - 1) + 1
                    nc.vector.tensor_copy(
                        out=colsum_sb[0:hi, b * W : (b + 1) * W],
                        in_=colps[0:hi, :],
                    )
                    nc.sync.dma_start(
                        out=colsum_d[3 * b : 3 * b + nb, :],
                        in_=colsum_sb[0 : hi : 32, b * W : (b + 1) * W],
                    )
            nc.vector.reduce_sum(
                rowfinal[:, mt : mt + 1],
                rowsums_sb[:, mt * GN : (mt + 1) * GN],
                axis=mybir.AxisListType.X,
            )
            # diag work spread through the main loop (fills DVE idle time)
            mn = diagpool.tile([P, D], f32, tag="mn")
            sn = diagpool.tile([P, D], f32, tag="sn")
            prod = diagpool.tile([P, D], f32, tag="prod")
            nc.sync.dma_start(out=mn, in_=mnat_d[mt * P : (mt + 1) * P, :])
            nc.sync.dma_start(out=sn, in_=snat_d[mt * P : (mt + 1) * P, :])
            nc.vector.tensor_mul(prod, mn, sn)
            nc.vector.reduce_sum(
                diagfinal[:, mt : mt + 1], prod, axis=mybir.AxisListType.X
            )

        nc.sync.dma_start(out=rowsum_d[:, :], in_=rowfinal)
        nc.sync.dma_start(out=diag_d[:, :], in_=diagfinal)

    nc.compile()
    return nc


def _get_nc(shift: float, scale: float):
    key = (shift, scale)
    if key not in _nc_cache:
        _nc_cache[key] = _build(shift, scale)
    return _nc_cache[key]


def _interleave_fp8(x: np.ndarray) -> np.ndarray:
    """x [N, D] f32 -> [P, KC, N] fp8 with x8[p, j, c] = x[c, j*128 + p]."""
    xq = x.astype(FP8)
    xT = np.ascontiguousarray(xq.T)  # [D, N]
    return np.ascontiguousarray(xT.reshape(KC, P, -1).transpose(1, 0, 2))


def run(inputs: dict, trace: bool = False):
    m = np.asarray(inputs["modality_features"], dtype=np.float32)
    s = np.asarray(inputs["sequence_features"], dtype=np.float32)
    scale = float(np.asarray(inputs["logit_scale"], dtype=np.float32))
    assert m.shape == (B, D) and s.shape == (B, D)

    shift = float(6.0 * abs(scale) * math.sqrt(D))
    nc = _get_nc(shift, scale)

    ms = m * np.float32(scale)
    s8 = _interleave_fp8(s)

    in_maps = []
    for c in range(NCORES):
        r = slice(c * ROWS, (c + 1) * ROWS)
        in_maps.append(
            {
                "m8": _interleave_fp8(m[r]),
                "s8": s8,
                "mnat": np.ascontiguousarray(ms[r]),
                "snat": np.ascontiguousarray(s[r]),
            }
        )

    res = run_bass_kernel_spmd(nc, in_maps, list(range(NCORES)), trace=trace)

    rowsum = np.concatenate(
        [r["rowsum"].T.reshape(-1) for r in res.results]
    ).astype(np.float64)
    diag = np.concatenate([r["diag"].T.reshape(-1) for r in res.results]).astype(
        np.float64
    )
    colsum = np.zeros(B, dtype=np.float64)
    for r in res.results:
        colsum += r["colsum"].astype(np.float64).reshape(B)

    rowlse = shift + np.log(rowsum)
    collse = shift + np.log(colsum)
    loss = np.mean(0.5 * (rowlse + collse) - diag)
    return np.asarray(loss, dtype=np.float32), res


def kernel(**inputs) -> np.ndarray:
    out, _ = run(inputs, trace=False)
    return out


# revision 4
# speedup vs baseline: 1.0094x; 1.0094x over previous
"""CLIP (InfoNCE) loss kernel for Trainium2, 8 NeuronCores.

loss = 0.5*(ce_m + ce_s) where
  ce_m = mean_i( LSE_j(l[i,:]) - l[i,i] ),  ce_s = mean_j( LSE_i(l[:,j]) - l[j,j] )
  l = logit_scale * (m @ s.T),  B=16384, D=256.

Data parallel on batch rows, 8 cores; core c owns rows [c*2048, (c+1)*2048)
of m and sees the full s.

Per core:
  - Features are quantized (UNSCALED) to fp8 e4m3 in a k-interleaved layout
    [128, 2, N]; main logits tiles use DoubleRow fp8 matmuls (K=256 fused in
    one PE pass, ~2x bf16 FLOPs).  logit_scale is applied inside the ACT
    affine (exp(scale*l - shift)), so quantization never clips.
  - mt-outer / g-inner tiling: 16 row-tiles x 8 column groups of [128, 2048].
    One ScalarE exp per group tile (PSUM f32 -> SBUF bf16) with fused
    accum_out producing the per-row partial sums (computed pre-rounding in
    f32) -- ScalarE is the bottleneck engine and runs ~94% occupied.
  - Column sums: per-group bf16 accumulators acc_g += E on DVE (idle
    otherwise); at mt=0 ACT writes acc_g directly; the final row-tile's E
    skips the add (kept in e15_g) so the tail never waits on DVE.  Ones-
    vector matmuls then reduce acc_g and e15_g across partitions (PSUM-
    accumulated pairs), batched 3 groups per borrowed PSUM slot on PE
    column strips 0/32/64 (quad 3 unusable).
  - diag l[i,i] is exact f32: row-dot of natural-layout scaled shards
    (DVE mul+reduce), spread through the main loop.
  - host merges per-core partials in float64:
      rowLSE = SHIFT + log(rowsum); colLSE = SHIFT + log(sum_c colsum_c)
      loss = mean(0.5*(rowLSE + colLSE) - diag)

SHIFT = 6*|scale|*sqrt(D) (a ~6-sigma bound on logits ~ N(0, scale^2 D)):
exp never overflows, and underflow to 0 only hits terms ~e^-80 below the
row/col max -- far below f32 relative precision.  fp8 quantization of the
inputs perturbs the loss by ~7e-4 relative (tolerance 2e-2): LSE is
max-dominated, the exact-diag term is computed in f32, and quantization
noise on 256-term dots is ~0.5 absolute on logits with sigma=16.
"""

import math
from contextlib import ExitStack

import numpy as np
import ml_dtypes

import concourse.bacc as bacc
import concourse.tile as tile
from concourse import mybir
from concourse.bass_utils import run_bass_kernel_spmd

FP8 = ml_dtypes.float8_e4m3

B = 16384
D = 256
NCORES = 8
ROWS = B // NCORES          # 2048 rows per core
P = 128
MT = ROWS // P              # 16 row-tiles
KC = D // P                 # 2 k-chunks (fused by DoubleRow)
W = 2048                    # column group width (4 psum banks f32)
GN = B // W                 # 8 column groups
SUBW = 512                  # matmul free dim (one psum bank)
NSUB = W // SUBW            # 4

f32 = mybir.dt.float32
bf16 = mybir.dt.bfloat16
fp8 = mybir.dt.float8e4

_nc_cache: dict = {}


def _build(shift: float, scale: float) -> "bacc.Bacc":
    nc = bacc.Bacc(trn_type="TRN2")

    m8_d = nc.dram_tensor("m8", [P, KC, ROWS], fp8, kind="ExternalInput")
    s8_d = nc.dram_tensor("s8", [P, KC, B], fp8, kind="ExternalInput")
    mnat_d = nc.dram_tensor("mnat", [ROWS, D], f32, kind="ExternalInput")
    snat_d = nc.dram_tensor("snat", [ROWS, D], f32, kind="ExternalInput")

    rowsum_d = nc.dram_tensor("rowsum", [P, MT], f32, kind="ExternalOutput")
    diag_d = nc.dram_tensor("diag", [P, MT], f32, kind="ExternalOutput")
    colsum_d = nc.dram_tensor("colsum", [GN, W], f32, kind="ExternalOutput")

    with ExitStack() as ctx:
        tc = ctx.enter_context(tile.TileContext(nc))
        singles = ctx.enter_context(tc.tile_pool(name="singles", bufs=1))
        epool = ctx.enter_context(tc.tile_pool(name="epool", bufs=6))
        diagpool = ctx.enter_context(tc.tile_pool(name="diagpool", bufs=4))
        mainps = ctx.enter_context(tc.tile_pool(name="mainps", bufs=2, space="PSUM"))

        # ramp: first row-tile of m8 and first chunk of s8[0] land first
        m8_sb = singles.tile([P, KC, ROWS], fp8, tag="m8")
        nc.sync.dma_start(out=m8_sb[:, :, 0:P], in_=m8_d[:, :, 0:P])
        s8_sb = [
            singles.tile([P, KC, W], fp8, name=f"s8_{g}", tag=f"s8_{g}")
            for g in range(GN)
        ]
        for q in range(4):
            nc.sync.dma_start(
                out=s8_sb[0][:, :, q * SUBW : (q + 1) * SUBW],
                in_=s8_d[:, :, q * SUBW : (q + 1) * SUBW],
            )
        nc.sync.dma_start(out=m8_sb[:, :, P:ROWS], in_=m8_d[:, :, P:ROWS])
        for g in range(1, GN):
            nc.sync.dma_start(out=s8_sb[g], in_=s8_d[:, :, g * W : (g + 1) * W])

        ones = singles.tile([P, 1], bf16, tag="ones")
        nc.vector.memset(ones, 1.0)
        negshift = singles.tile([P, 1], f32, tag="negshift")
        nc.vector.memset(negshift, -shift)

        rowsums_sb = singles.tile([P, MT * GN], f32, tag="rowsums")
        rowfinal = singles.tile([P, MT], f32, tag="rowfinal")
        colsum_sb = singles.tile([P, 3 * W], f32, tag="colsum")
        diagfinal = singles.tile([P, MT], f32, tag="diagfinal")
        accs = [
            singles.tile([P, W], bf16, name=f"acc_{g}", tag=f"acc_{g}")
            for g in range(GN)
        ]
        e15 = [
            singles.tile([P, W], bf16, name=f"e15_{g}", tag=f"e15_{g}")
            for g in range(GN)
        ]

        for mt in range(MT):
            for g in range(GN):
                ps = mainps.tile([P, W], f32, tag="ps")
                for sub in range(NSUB):
                    nc.tensor.matmul(
                        ps[:, sub * SUBW : (sub + 1) * SUBW],
                        lhsT=m8_sb[:, :, mt * P : (mt + 1) * P],
                        rhs=s8_sb[g][:, :, sub * SUBW : (sub + 1) * SUBW],
                        start=True,
                        stop=True,
                        perf_mode=mybir.MatmulPerfMode.DoubleRow,
                    )
                slot = mt * GN + g
                # ~30% of row-tiles compute their rowsum on DVE instead of
                # the ACT accumulator: balances ACT (bottleneck, -187ns per
                # offloaded instr) against DVE slack (+2.6us per reduce)
                offload = mt > 0 and (slot * 997) % 128 < 38
                if mt == 0:
                    nc.scalar.activation(
                        accs[g], ps, mybir.ActivationFunctionType.Exp,
                        bias=negshift[:, 0:1], scale=scale,
                        accum_out=rowsums_sb[:, slot : slot + 1],
                    )
                elif offload:
                    last = mt == MT - 1
                    e = e15[g] if last else epool.tile([P, W], bf16)
                    nc.scalar.activation(
                        e, ps, mybir.ActivationFunctionType.Exp,
                        bias=negshift[:, 0:1], scale=scale,
                    )
                    nc.vector.reduce_sum(
                        rowsums_sb[:, slot : slot + 1], e,
                        axis=mybir.AxisListType.X,
                    )
                    if not last:
                        nc.vector.tensor_add(accs[g], e, accs[g])
                else:
                    last = mt == MT - 1
                    e = e15[g] if last else epool.tile([P, W], bf16)
                    nc.scalar.activation(
                        e, ps, mybir.ActivationFunctionType.Exp,
                        bias=negshift[:, 0:1], scale=scale,
                        accum_out=rowsums_sb[:, slot : slot + 1],
                    )
                    if not last:
                        nc.vector.tensor_add(accs[g], e, accs[g])
                if mt == MT - 1 and g in (2, 5, 7):
                    # colsum batch: acc_g final after add(15, g); borrow a
                    # mainps slot, one PE column strip (0/32/64) per group
                    batch = {2: (0, 1, 2), 5: (3, 4, 5), 7: (6, 7)}[g]
                    b = {2: 0, 5: 1, 7: 2}[g]
                    colps = mainps.tile([P, W], f32, tag="ps", name="colps")
                    for idx, gg in enumerate(batch):
                        row = 32 * idx
                        for sub in range(NSUB):
                            nc.tensor.matmul(
                                colps[row : row + 1, sub * SUBW : (sub + 1) * SUBW],
                                lhsT=ones,
                                rhs=accs[gg][:, sub * SUBW : (sub + 1) * SUBW],
                                start=True,
                                stop=False,
                            )
                            nc.tensor.matmul(
                                colps[row : row + 1, sub * SUBW : (sub + 1) * SUBW],
                                lhsT=ones,
                                rhs=e15[gg][:, sub * SUBW : (sub + 1) * SUBW],
                                start=False,
                                stop=True,
                            )
                    nb = len(batch)
                    hi = 32 * (nb - 1) + 1
                    nc.vector.tensor_copy(
                        out=colsum_sb[0:hi, b * W : (b + 1) * W],
                        in_=colps[0:hi, :],
                    )
                    nc.sync.dma_start(
                        out=colsum_d[3 * b : 3 * b + nb, :],
                        in_=colsum_sb[0 : hi : 32, b * W : (b + 1) * W],
                    )
            nc.vector.reduce_sum(
                rowfinal[:, mt : mt + 1],
                rowsums_sb[:, mt * GN : (mt + 1) * GN],
                axis=mybir.AxisListType.X,
            )
            # diag work spread through the main loop (fills DVE idle time)
            mn = diagpool.tile([P, D], f32, tag="mn")
            sn = diagpool.tile([P, D], f32, tag="sn")
            prod = diagpool.tile([P, D], f32, tag="prod")
            nc.sync.dma_start(out=mn, in_=mnat_d[mt * P : (mt + 1) * P, :])
            nc.sync.dma_start(out=sn, in_=snat_d[mt * P : (mt + 1) * P, :])
            nc.vector.tensor_mul(prod, mn, sn)
            nc.vector.reduce_sum(
                diagfinal[:, mt : mt + 1], prod, axis=mybir.AxisListType.X
            )

        nc.sync.dma_start(out=rowsum_d[:, :], in_=rowfinal)
        nc.sync.dma_start(out=diag_d[:, :], in_=diagfinal)

    nc.compile()
    return nc


def _get_nc(shift: float, scale: float):
    key = (shift, scale)
    if key not in _nc_cache:
        _nc_cache[key] = _build(shift, scale)
    return _nc_cache[key]


def _interleave_fp8(x: np.ndarray) -> np.ndarray:
    """x [N, D] f32 -> [P, KC, N] fp8 with x8[p, j, c] = x[c, j*128 + p]."""
    xq = x.astype(FP8)
    xT = np.ascontiguousarray(xq.T)  # [D, N]
    return np.ascontiguousarray(xT.reshape(KC, P, -1).transpose(1, 0, 2))


def run(inputs: dict, trace: bool = False):
    m = np.asarray(inputs["modality_features"], dtype=np.float32)
    s = np.asarray(inputs["sequence_features"], dtype=np.float32)
    scale = float(np.asarray(inputs["logit_scale"], dtype=np.float32))
    assert m.shape == (B, D) and s.shape == (B, D)

    shift = float(6.0 * abs(scale) * math.sqrt(D))
    nc = _get_nc(shift, scale)

    ms = m * np.float32(scale)
    s8 = _interleave_fp8(s)

    in_maps = []
    for c in range(NCORES):
        r = slice(c * ROWS, (c + 1) * ROWS)
        in_maps.append(
            {
                "m8": _interleave_fp8(m[r]),
                "s8": s8,
                "mnat": np.ascontiguousarray(ms[r]),
                "snat": np.ascontiguousarray(s[r]),
            }
        )

    res = run_bass_kernel_spmd(nc, in_maps, list(range(NCORES)), trace=trace)

    rowsum = np.concatenate(
        [r["rowsum"].T.reshape(-1) for r in res.results]
    ).astype(np.float64)
    diag = np.concatenate([r["diag"].T.reshape(-1) for r in res.results]).astype(
        np.float64
    )
    colsum = np.zeros(B, dtype=np.float64)
    for r in res.results:
        colsum += r["colsum"].astype(np.float64).reshape(B)

    rowlse = shift + np.log(rowsum)
    collse = shift + np.log(colsum)
    loss = np.mean(0.5 * (rowlse + collse) - diag)
    return np.asarray(loss, dtype=np.float32), res


def kernel(**inputs) -> np.ndarray:
    out, _ = run(inputs, trace=False)
    return out
